# revision 4
# baseline (speedup 1.0000x reference)
"""Bass/Trainium2 kernel for masked dot-product attention.

Math (per batch b):
  scores = q @ k^T / sqrt(D); masked positions (j >= valid[i]) -> 1e-6
  weights = softmax(scores, -1); out = weights @ v

Strategy (v3):
  - Shard batch dim B=16 across 8 cores (2 batches/core), SPMD program.
  - Host: sort rows of each batch by valid[i] -> monotone mask staircase;
    fully-masked (i,j)-tiles are skipped; their exact contribution
    exp(1e-6)*(suffix sums of v) is added via a host-built correction
    tensor (identity matmul into the same accumulator).
  - Device: S^T tiles [j=128 part, i<=512 free] on PE in fp16, with q
    pre-scaled by sqrt(A), k by sqrt(A) (A = 1024*0.125*log2 e) and a
    65th contraction row adding B = 15360 = 1024*15: the psum value IS
    the fp16 bit pattern of e^{s/8} (Schraudolph).  exp splits between:
      * ACT: exact exp (scale/bias fold A,B away), fp16 out
      * DVE: convert-to-int16 (round-to-nearest) + a 3-op quadratic
        mantissa correction, all in 2x/4x-eligible 16-bit ops.
    The mask staircase is a {1,0} fp16 multiply (fused into the DVE
    path's first op; a separate cheap multiply after ACT tiles).
  - AV: out[i-subtile 128, 65] accumulates per j-tile with E stationary
    (moving dim 65 incl. ones column -> softmax denominator for free).
  - Normalize: DVE reciprocal of z + broadcast multiply -> fp16 out.
  - GPSIMD cannot touch PSUM; it only zero-fills e-tile gap rows.
  - PE p-state warmup: 8 throwaway matmuls during the input-DMA window.
"""

import numpy as np

import concourse.bass as bass
import concourse.tile as tile
import concourse.mybir as mybir
from concourse import bacc
from concourse.bass_utils import run_bass_kernel_spmd
from concourse.masks import make_identity

B, N, D = 16, 2048, 64
NCORES = 8
NB = B // NCORES          # batches per core
IW = 512                  # i-range width
NI = N // IW              # 4 i-ranges
JW = 128                  # j tile width
NJ = N // JW              # 16 j tiles
SUB = 128                 # AV i-subtile
DV = D + 1

f16 = mybir.dt.float16
f32 = mybir.dt.float32
i16 = mybir.dt.int16

A_TRICK = 184.664955          # 1024 * 0.125 * log2(e)
SQ_A = float(np.sqrt(A_TRICK))
B_OFF = 15360.0               # 1024 * 15 (fp16 exponent bias)
SC_ACT = 0.125 / A_TRICK
E6 = float(np.exp(np.float32(1e-6)))

# quadratic fit of c(f) = 2^f/(1+f) on [0,1]; e = eh*c2*(f^2 + r1 f + r0),
# the common factor c2 cancels in the softmax ratio but is ALSO applied to
# the ACT path (bias shift) for cross-path consistency.
_ff = np.linspace(0.0, 1.0, 401)
_C2, _C1, _C0 = [float(x) for x in np.polyfit(_ff, 2.0**_ff / (1.0 + _ff), 2)]
R1 = _C1 / _C2
R0 = _C0 / _C2
LN_C2 = float(np.log(_C2))

LOOKAHEAD = 3


class Plan:
    def __init__(self):
        self.taus = [[] for _ in range(NI)]   # per r: list of tau dicts
        self.m16_w = 0


def _classify(t_sorted):
    plan = Plan()
    off = 0
    for r in range(NI):
        tw = t_sorted[:, r * IW:(r + 1) * IW]  # [B, IW] sorted ascending
        for tau in range(NJ):
            jlo, jhi = JW * tau, JW * (tau + 1)
            n_le = (tw <= jlo).sum(axis=1)
            n_lt = (tw < jhi).sum(axis=1)
            lo = int(n_le.min())
            if lo >= IW:
                break
            mhi = int(n_lt.max())
            x0s = lo & ~15           # S-matmul / exp start (16-aligned)
            x0a = lo & ~127          # AV-subtile start (128-aligned)
            w1 = max(mhi, x0s)
            ti = {
                "tau": tau, "lo": lo, "x0s": x0s, "x0a": x0a, "w1": w1,
                "m_off": None,
            }
            if w1 > x0s:
                ti["m_off"] = off
                off += w1 - x0s
            plan.taus[r].append(ti)
    plan.m16_w = max(off, 16)
    return plan


class _Greedy:
    """Balance exp tiles between ACT (exact exp) and DVE (trick+fix)."""

    def __init__(self):
        self.load = {"ACT": 0.0, "DVE": 0.0}

    def pick_exp(self, n, wb):
        """n = exp width, wb = masked-boundary width (0 if clean)."""
        act_c = (n + 222) * 0.8333
        act_dve = (0.5 * wb + 58) * 1.0417 if wb else 0.0
        dve_c = (n + 120) * 1.0417 + 3 * (0.25 * n + 58) * 1.0417
        t_act = max(self.load["ACT"] + act_c, self.load["DVE"] + act_dve)
        t_dve = max(self.load["ACT"], self.load["DVE"] + dve_c)
        if t_act <= t_dve:
            self.load["ACT"] += act_c
            self.load["DVE"] += act_dve
            return "ACT"
        self.load["DVE"] += dve_c
        return "DVE"

    def add_dve(self, ns):
        self.load["DVE"] += ns


def _build_program(plan):
    nc = bacc.Bacc("TRN2", target_bir_lowering=False, debug=False)

    qT = nc.dram_tensor("qT", [NB, DV, N], f16, kind="ExternalInput").ap()
    kT = nc.dram_tensor("kT", [NB, DV, N], f16, kind="ExternalInput").ap()
    vw = nc.dram_tensor("vw", [NB, 128, NJ, DV], f16, kind="ExternalInput").ap()
    corr = nc.dram_tensor("corr", [NB, 128, NI, 4, DV], f16,
                          kind="ExternalInput").ap()
    m16 = nc.dram_tensor("m16", [NB, 128, plan.m16_w], f16,
                         kind="ExternalInput").ap()
    out = nc.dram_tensor("out", [NB, 128, NI, 4, D], f16,
                         kind="ExternalOutput").ap()

    gr = _Greedy()

    with tile.TileContext(nc, trace_sim=False) as tc:
        with (
            tc.tile_pool(name="consts", bufs=1) as consts,
            tc.tile_pool(name="sb_qk", bufs=2) as sb_qk,
            tc.tile_pool(name="sb_vc", bufs=2) as sb_vc,
            tc.tile_pool(name="sb_e", bufs=6) as sb_e,
            tc.tile_pool(name="sb_eh", bufs=4) as sb_eh,
            tc.tile_pool(name="sb_t", bufs=4) as sb_t,
            tc.tile_pool(name="sb_o", bufs=3) as sb_o,
            tc.tile_pool(name="sb_z", bufs=3) as sb_z,
            tc.tile_pool(name="ps_s", bufs=6, space="PSUM") as ps_s,
            tc.tile_pool(name="ps_acc", bufs=2, space="PSUM") as ps_acc,
        ):
            ident = consts.tile([128, 128], f32)
            make_identity(nc, ident)
            identh = consts.tile([128, 128], f16)
            nc.vector.tensor_copy(identh, ident)
            bias_t = consts.tile([128, 1], f32)
            nc.vector.memset(bias_t, float(-B_OFF * SC_ACT + LN_C2))
            wsrc = consts.tile([128, 512], f16)
            nc.vector.memset(wsrc, 0.125)

            # p-state warmup; also initializes all 6 S-psum buffers
            for wu in range(8):
                ps_w = ps_s.tile([128, 512], f32, tag="s", name=f"wu{wu}")
                nc.tensor.matmul(ps_w, identh, wsrc, start=True, stop=True)

            # input DMAs for both batches up-front (sync/SP queue)
            ins = []
            for bi in range(NB):
                q_sb = sb_qk.tile([DV, N], f16, tag="q")
                k_sb = sb_qk.tile([DV, N], f16, tag="k")
                nc.sync.dma_start(out=q_sb, in_=qT[bi])
                nc.sync.dma_start(out=k_sb, in_=kT[bi])
                corr_sb = sb_vc.tile([128, NI, 4, DV], f16, tag="corr")
                nc.sync.dma_start(out=corr_sb, in_=corr[bi])
                vw_sb = sb_vc.tile([128, NJ, DV], f16, tag="vw")
                nc.sync.dma_start(out=vw_sb, in_=vw[bi])
                m_sb = sb_vc.tile([128, plan.m16_w], f16, tag="m16")
                nc.sync.dma_start(out=m_sb, in_=m16[bi])
                ins.append((q_sb, k_sb, vw_sb, corr_sb, m_sb))

            def emit_exp(eng, ps_t, e_t, ti, bi, r):
                """exp over [x0s, 512) of tile ti; zero-fill [x0a, x0s)."""
                x0, w1, tau = ti["x0s"], ti["w1"], ti["tau"]
                wb = w1 - x0
                m_sb = ins[bi][4]
                nm = f"{bi}_{r}_{tau}"
                if ti["x0a"] < x0:
                    nc.gpsimd.memset(e_t[:, ti["x0a"]:x0], 0.0)
                if eng == "ACT":
                    nc.scalar.activation(
                        e_t[:, x0:], ps_t[:, x0:],
                        mybir.ActivationFunctionType.Exp,
                        bias=bias_t[:, 0:1], scale=float(SC_ACT))
                    if wb:
                        mo = ti["m_off"]
                        nc.vector.tensor_tensor(
                            out=e_t[:, x0:w1], in0=e_t[:, x0:w1],
                            in1=m_sb[:, mo:mo + wb],
                            op=mybir.AluOpType.mult)
                    return
                # DVE: trick + quadratic mantissa fix
                eh = sb_eh.tile([128, IW], f16, tag="eh", name=f"eh_{nm}")
                if wb:
                    mo = ti["m_off"]
                    nc.vector.scalar_tensor_tensor(
                        out=eh[:, x0:].bitcast(i16), in0=ps_t[:, x0:],
                        scalar=0.0, in1=m_sb[:, mo:mo + (IW - x0)],
                        op0=mybir.AluOpType.max, op1=mybir.AluOpType.mult)
                else:
                    nc.vector.tensor_scalar(
                        out=eh[:, x0:].bitcast(i16), in0=ps_t[:, x0:],
                        scalar1=0.0, scalar2=None, op0=mybir.AluOpType.max)
                fb = sb_t.tile([128, IW], f16, tag="fb", name=f"fb_{nm}")
                nc.vector.tensor_scalar(
                    out=fb[:, x0:], in0=eh[:, x0:].bitcast(i16),
                    scalar1=1024.0, scalar2=float(1.0 / 1024.0),
                    op0=mybir.AluOpType.mod, op1=mybir.AluOpType.mult)
                w_t = sb_t.tile([128, IW], f16, tag="w", name=f"w_{nm}")
                nc.vector.scalar_tensor_tensor(
                    out=w_t[:, x0:], in0=fb[:, x0:], scalar=float(R1),
                    in1=fb[:, x0:], op0=mybir.AluOpType.add,
                    op1=mybir.AluOpType.mult)
                nc.vector.scalar_tensor_tensor(
                    out=e_t[:, x0:], in0=w_t[:, x0:], scalar=float(R0),
                    in1=eh[:, x0:], op0=mybir.AluOpType.add,
                    op1=mybir.AluOpType.mult)

            for bi in range(NB):
                q_sb, k_sb, vw_sb, corr_sb, m_sb = ins[bi]
                for r in range(NI - 1, -1, -1):
                    taus = plan.taus[r]
                    pacc = ps_acc.tile([128, 4, DV], f32, tag="acc",
                                       name=f"acc_{bi}_{r}")
                    nc.tensor.matmul(
                        pacc, identh, corr_sb[:, r],
                        start=True, stop=False)

                    n_av = sum(4 - ti["x0a"] // SUB for ti in taus)
                    e_tiles = [None] * len(taus)
                    av_done = 0

                    def emit_av(idx):
                        nonlocal av_done
                        ti = taus[idx]
                        e_t = e_tiles[idx]
                        for ib in range(ti["x0a"] // SUB, 4):
                            av_done += 1
                            nc.tensor.matmul(
                                pacc[:, ib, :],
                                e_t[:, ib * SUB:(ib + 1) * SUB],
                                vw_sb[:, ti["tau"], :],
                                start=False, stop=(av_done == n_av))

                    for idx, ti in enumerate(taus):
                        tau = ti["tau"]
                        ps_t = ps_s.tile([128, IW], f32, tag="s",
                                         name=f"s_{bi}_{r}_{tau}")
                        nc.tensor.matmul(
                            ps_t[:, ti["x0s"]:],
                            k_sb[:, tau * JW:(tau + 1) * JW],
                            q_sb[:, r * IW + ti["x0s"]:(r + 1) * IW],
                            start=True, stop=True)
                        e_t = sb_e.tile([128, IW], f16, tag="e",
                                        name=f"e_{bi}_{r}_{tau}")
                        e_tiles[idx] = e_t
                        eng = gr.pick_exp(IW - ti["x0s"], ti["w1"] - ti["x0s"])
                        emit_exp(eng, ps_t, e_t, ti, bi, r)
                        if idx >= LOOKAHEAD:
                            emit_av(idx - LOOKAHEAD)
                    for idx in range(max(0, len(taus) - LOOKAHEAD),
                                     len(taus)):
                        emit_av(idx)

                    # normalize on DVE
                    zinv = sb_z.tile([128, 4], f32, tag="z",
                                     name=f"z_{bi}_{r}")
                    nc.vector.reciprocal(zinv, pacc[:, :, D])
                    osb = sb_o.tile([128, 4, D], f16, tag="o",
                                    name=f"o_{bi}_{r}")
                    nc.vector.tensor_tensor(
                        out=osb, in0=pacc[:, :, 0:D],
                        in1=zinv.unsqueeze(2).broadcast_to([128, 4, D]),
                        op=mybir.AluOpType.mult)
                    gr.add_dve(129 + (256 + 120) * 1.0417)
                    nc.sync.dma_start(out=out[bi][:, r], in_=osb)
    nc.compile()
    return nc


def _host_prep(q, k, v, valid):
    t = np.clip(np.asarray(valid).astype(np.int64), 0, N)
    perm = np.argsort(t, axis=1, kind="stable")
    t_s = np.take_along_axis(t, perm, axis=1)
    q_s = np.take_along_axis(np.asarray(q, np.float32), perm[..., None],
                             axis=1)
    plan = _classify(t_s)

    qT = np.empty((B, DV, N), np.float16)
    qT[:, 0:D] = np.swapaxes(q_s * SQ_A, 1, 2).astype(np.float16)
    qT[:, D] = 128.0
    kT = np.empty((B, DV, N), np.float16)
    kT[:, 0:D] = np.swapaxes(np.asarray(k, np.float32) * SQ_A, 1, 2
                             ).astype(np.float16)
    kT[:, D] = 120.0

    v32 = np.asarray(v, np.float32)
    vwt = np.empty((B, 128, NJ, DV), np.float16)
    vwt[:, :, :, 0:D] = np.swapaxes(
        v32.reshape(B, NJ, 128, D), 1, 2).astype(np.float16)
    vwt[:, :, :, D] = 1.0

    ss = np.zeros((B, N + 1, D), np.float64)
    ss[:, :-1] = np.cumsum(v32[:, ::-1, :].astype(np.float64),
                           axis=1)[:, ::-1, :]
    ssg = np.take_along_axis(ss, t_s[..., None], axis=1)   # [B, N, D]
    cnt = (N - t_s).astype(np.float64)                     # [B, N]
    corr = np.empty((B, N, DV), np.float64)
    corr[:, :, 0:D] = ssg * E6
    corr[:, :, D] = cnt * E6
    corrt = np.ascontiguousarray(
        corr.reshape(B, NI, 4, 128, DV).transpose(0, 3, 1, 2, 4)
    ).astype(np.float16)

    m16v = np.zeros((B, 128, plan.m16_w), np.float16)
    jj = np.arange(128)
    for r in range(NI):
        for ti in plan.taus[r]:
            if ti["m_off"] is None:
                continue
            x0, w1, tau = ti["x0s"], ti["w1"], ti["tau"]
            tloc = t_s[:, r * IW + x0: r * IW + w1]          # [B, w]
            mloc = tloc[:, None, :] > (JW * tau + jj)[None, :, None]
            m16v[:, :, ti["m_off"]:ti["m_off"] + (w1 - x0)] = mloc
    return plan, perm, qT, kT, vwt, corrt, m16v


LAST = {}


def kernel(q, k, v, valid, _trace=False):
    plan, perm, qT, kT, vwt, corrt, m16v = _host_prep(q, k, v, valid)
    nc = _build_program(plan)

    in_maps = []
    for c in range(NCORES):
        sl = slice(c * NB, (c + 1) * NB)
        in_maps.append({
            "qT": np.ascontiguousarray(qT[sl]),
            "kT": np.ascontiguousarray(kT[sl]),
            "vw": np.ascontiguousarray(vwt[sl]),
            "corr": np.ascontiguousarray(corrt[sl]),
            "m16": np.ascontiguousarray(m16v[sl]),
        })
    res = run_bass_kernel_spmd(nc, in_maps, list(range(NCORES)),
                               trace=_trace)
    LAST["res"] = res
    LAST["nc"] = nc

    out = np.empty((B, N, D), np.float32)
    for c in range(NCORES):
        o = res.results[c]["out"]          # [NB, 128, NI, 4, D] fp16
        for bi in range(NB):
            b = c * NB + bi
            o_sorted = o[bi].transpose(1, 2, 0, 3).reshape(N, D)
            out[b, perm[b]] = o_sorted.astype(np.float32)
    return out


# revision 5
# speedup vs baseline: 1.5377x; 1.5377x over previous
"""Bass/Trainium2 kernel for masked dot-product attention.

Math (per batch b):
  scores = q @ k^T / sqrt(D); masked positions (j >= valid[i]) -> 1e-6
  weights = softmax(scores, -1); out = weights @ v

Strategy (v3):
  - Shard batch dim B=16 across 8 cores (2 batches/core), SPMD program.
  - Host: sort rows of each batch by valid[i] -> monotone mask staircase;
    fully-masked (i,j)-tiles are skipped; their exact contribution
    exp(1e-6)*(suffix sums of v) is added via a host-built correction
    tensor (identity matmul into the same accumulator).
  - Device: S^T tiles [j=128 part, i<=512 free] on PE in fp16, with q
    pre-scaled by sqrt(A), k by sqrt(A) (A = 1024*0.125*log2 e) and a
    65th contraction row adding B = 15360 = 1024*15: the psum value IS
    the fp16 bit pattern of e^{s/8} (Schraudolph).  exp splits between:
      * ACT: exact exp (scale/bias fold A,B away), fp16 out
      * DVE: convert-to-int16 (round-to-nearest) + a 3-op quadratic
        mantissa correction, all in 2x/4x-eligible 16-bit ops.
    The mask staircase is a {1,0} fp16 multiply (fused into the DVE
    path's first op; a separate cheap multiply after ACT tiles).
  - AV: out[i-subtile 128, 65] accumulates per j-tile with E stationary
    (moving dim 65 incl. ones column -> softmax denominator for free).
  - Normalize: DVE reciprocal of z + broadcast multiply -> fp16 out.
  - GPSIMD cannot touch PSUM; it only zero-fills e-tile gap rows.
  - PE p-state warmup: 8 throwaway matmuls during the input-DMA window.
"""

import numpy as np

import concourse.bass as bass
import concourse.tile as tile
import concourse.mybir as mybir
from concourse import bacc
from concourse.bass_utils import run_bass_kernel_spmd
from concourse.masks import make_identity

B, N, D = 16, 2048, 64
NCORES = 8
NB = B // NCORES          # batches per core
IW = 512                  # i-range width
NI = N // IW              # 4 i-ranges
JW = 128                  # j tile width
NJ = N // JW              # 16 j tiles
SUB = 128                 # AV i-subtile
DV = D + 1

f16 = mybir.dt.float16
f32 = mybir.dt.float32
i16 = mybir.dt.int16

A_TRICK = 184.664955          # 1024 * 0.125 * log2(e)
SQ_A = float(np.sqrt(A_TRICK))
B_OFF = 15360.0               # 1024 * 15 (fp16 exponent bias)
SC_ACT = 0.125 / A_TRICK
E6 = float(np.exp(np.float32(1e-6)))

# quadratic fit of c(f) = 2^f/(1+f) on [0,1]; e = eh*c2*(f^2 + r1 f + r0),
# the common factor c2 cancels in the softmax ratio but is ALSO applied to
# the ACT path (bias shift) for cross-path consistency.
_ff = np.linspace(0.0, 1.0, 401)
_C2, _C1, _C0 = [float(x) for x in np.polyfit(_ff, 2.0**_ff / (1.0 + _ff), 2)]
R1 = _C1 / _C2
R0 = _C0 / _C2
LN_C2 = float(np.log(_C2))

LOOKAHEAD = 3
ACT_W = 1.0


class Plan:
    def __init__(self):
        self.taus = [[] for _ in range(NI)]   # per r: list of tau dicts
        self.m16_w = 0


def _classify(t_sorted):
    plan = Plan()
    off = 0
    for r in range(NI):
        tw = t_sorted[:, r * IW:(r + 1) * IW]  # [B, IW] sorted ascending
        for tau in range(NJ):
            jlo, jhi = JW * tau, JW * (tau + 1)
            n_le = (tw <= jlo).sum(axis=1)
            n_lt = (tw < jhi).sum(axis=1)
            lo = int(n_le.min())
            if lo >= IW:
                break
            mhi = int(n_lt.max())
            x0s = lo & ~15           # S-matmul / exp start (16-aligned)
            x0a = lo & ~127          # AV-subtile start (128-aligned)
            w1 = max(mhi, x0s)
            ti = {
                "tau": tau, "lo": lo, "x0s": x0s, "x0a": x0a, "w1": w1,
                "m_off": None,
            }
            if w1 > x0s:
                ti["m_off"] = off
                off += w1 - x0s
            plan.taus[r].append(ti)
    plan.m16_w = max(off, 16)
    return plan


class _Greedy:
    """Balance exp tiles between ACT (exact exp) and DVE (raw trick)."""

    def __init__(self, act_w=1.0):
        self.load = {"ACT": 0.0, "DVE": 0.0}
        self.act_w = act_w

    def pick_exp(self, n, wb):
        """n = exp width, wb = masked-boundary width (0 if clean)."""
        act_c = (n + 222) * 0.8333 * self.act_w
        act_dve = (0.5 * wb + 58) * 1.0417 if wb else 0.0
        dve_c = (n + 120) * 1.0417
        t_act = max(self.load["ACT"] + act_c, self.load["DVE"] + act_dve)
        t_dve = max(self.load["ACT"], self.load["DVE"] + dve_c)
        if t_act <= t_dve:
            self.load["ACT"] += act_c
            self.load["DVE"] += act_dve
            return "ACT"
        self.load["DVE"] += dve_c
        return "DVE"

    def pick_norm(self):
        self.load["DVE"] += 129.0
        act_c = 4 * (64 + 222) * 0.8333 * self.act_w
        dve_c = (256 + 120) * 1.0417
        if self.load["ACT"] + act_c <= self.load["DVE"] + dve_c:
            self.load["ACT"] += act_c
            return "ACT"
        self.load["DVE"] += dve_c
        return "DVE"


def _build_program(plan):
    nc = bacc.Bacc("TRN2", target_bir_lowering=False, debug=False)

    qT = nc.dram_tensor("qT", [NB, DV, N], f16, kind="ExternalInput").ap()
    kT = nc.dram_tensor("kT", [NB, DV, N], f16, kind="ExternalInput").ap()
    vw = nc.dram_tensor("vw", [NB, 128, NJ, DV], f16, kind="ExternalInput").ap()
    corr = nc.dram_tensor("corr", [NB, 128, NI, 4, DV], f16,
                          kind="ExternalInput").ap()
    m16 = nc.dram_tensor("m16", [NB, 128, plan.m16_w], f16,
                         kind="ExternalInput").ap()
    out = nc.dram_tensor("out", [NB, 128, NI, 4, D], f16,
                         kind="ExternalOutput").ap()

    gr = _Greedy(act_w=ACT_W)

    with tile.TileContext(nc, trace_sim=False) as tc:
        with (
            tc.tile_pool(name="consts", bufs=1) as consts,
            tc.tile_pool(name="sb_qk", bufs=2) as sb_qk,
            tc.tile_pool(name="sb_vc", bufs=2) as sb_vc,
            tc.tile_pool(name="sb_e", bufs=6) as sb_e,
            tc.tile_pool(name="sb_eh", bufs=4) as sb_eh,
            tc.tile_pool(name="sb_t", bufs=4) as sb_t,
            tc.tile_pool(name="sb_o", bufs=3) as sb_o,
            tc.tile_pool(name="sb_z", bufs=3) as sb_z,
            tc.tile_pool(name="ps_s", bufs=6, space="PSUM") as ps_s,
            tc.tile_pool(name="ps_acc", bufs=2, space="PSUM") as ps_acc,
        ):
            ident = consts.tile([128, 128], f32)
            make_identity(nc, ident)
            identh = consts.tile([128, 128], f16)
            nc.vector.tensor_copy(identh, ident)
            bias_t = consts.tile([128, 1], f32)
            nc.vector.memset(bias_t, float(-B_OFF * SC_ACT))
            wsrc = consts.tile([128, 512], f16)
            nc.vector.memset(wsrc, 0.125)

            # p-state warmup; also initializes all 6 S-psum buffers
            for wu in range(8):
                ps_w = ps_s.tile([128, 512], f32, tag="s", name=f"wu{wu}")
                nc.tensor.matmul(ps_w, identh, wsrc, start=True, stop=True)

            # input DMAs for both batches up-front (sync/SP queue)
            ins = []
            for bi in range(NB):
                q_sb = sb_qk.tile([DV, N], f16, tag="q")
                k_sb = sb_qk.tile([DV, N], f16, tag="k")
                nc.sync.dma_start(out=q_sb, in_=qT[bi])
                nc.sync.dma_start(out=k_sb, in_=kT[bi])
                corr_sb = sb_vc.tile([128, NI, 4, DV], f16, tag="corr")
                nc.sync.dma_start(out=corr_sb, in_=corr[bi])
                vw_sb = sb_vc.tile([128, NJ, DV], f16, tag="vw")
                nc.sync.dma_start(out=vw_sb, in_=vw[bi])
                m_sb = sb_vc.tile([128, plan.m16_w], f16, tag="m16")
                nc.sync.dma_start(out=m_sb, in_=m16[bi])
                ins.append((q_sb, k_sb, vw_sb, corr_sb, m_sb))

            def emit_exp(eng, ps_t, e_t, ti, bi, r):
                """exp over [x0s, 512) of tile ti; zero-fill [x0a, x0s)."""
                x0, w1, tau = ti["x0s"], ti["w1"], ti["tau"]
                wb = w1 - x0
                m_sb = ins[bi][4]
                if ti["x0a"] < x0:
                    nc.gpsimd.memset(e_t[:, ti["x0a"]:x0], 0.0)
                if eng == "ACT":
                    nc.scalar.activation(
                        e_t[:, x0:], ps_t[:, x0:],
                        mybir.ActivationFunctionType.Exp,
                        bias=bias_t[:, 0:1], scale=float(SC_ACT))
                    if wb:
                        mo = ti["m_off"]
                        nc.vector.tensor_tensor(
                            out=e_t[:, x0:w1], in0=e_t[:, x0:w1],
                            in1=m_sb[:, mo:mo + wb],
                            op=mybir.AluOpType.mult)
                    return
                # DVE: raw Schraudolph trick straight into the e tile
                if wb:
                    mo = ti["m_off"]
                    nc.vector.scalar_tensor_tensor(
                        out=e_t[:, x0:].bitcast(i16), in0=ps_t[:, x0:],
                        scalar=0.0, in1=m_sb[:, mo:mo + (IW - x0)],
                        op0=mybir.AluOpType.max, op1=mybir.AluOpType.mult)
                else:
                    nc.vector.tensor_scalar(
                        out=e_t[:, x0:].bitcast(i16), in0=ps_t[:, x0:],
                        scalar1=0.0, scalar2=None, op0=mybir.AluOpType.max)

            for bi in range(NB):
                q_sb, k_sb, vw_sb, corr_sb, m_sb = ins[bi]
                for r in range(NI - 1, -1, -1):
                    taus = plan.taus[r]
                    pacc = ps_acc.tile([128, 4, DV], f32, tag="acc",
                                       name=f"acc_{bi}_{r}")
                    nc.tensor.matmul(
                        pacc, identh, corr_sb[:, r],
                        start=True, stop=False)

                    n_av = sum(4 - ti["x0a"] // SUB for ti in taus)
                    e_tiles = [None] * len(taus)
                    av_done = 0

                    def emit_av(idx):
                        nonlocal av_done
                        ti = taus[idx]
                        e_t = e_tiles[idx]
                        for ib in range(ti["x0a"] // SUB, 4):
                            av_done += 1
                            nc.tensor.matmul(
                                pacc[:, ib, :],
                                e_t[:, ib * SUB:(ib + 1) * SUB],
                                vw_sb[:, ti["tau"], :],
                                start=False, stop=(av_done == n_av))

                    for idx, ti in enumerate(taus):
                        tau = ti["tau"]
                        ps_t = ps_s.tile([128, IW], f32, tag="s",
                                         name=f"s_{bi}_{r}_{tau}")
                        nc.tensor.matmul(
                            ps_t[:, ti["x0s"]:],
                            k_sb[:, tau * JW:(tau + 1) * JW],
                            q_sb[:, r * IW + ti["x0s"]:(r + 1) * IW],
                            start=True, stop=True)
                        e_t = sb_e.tile([128, IW], f16, tag="e",
                                        name=f"e_{bi}_{r}_{tau}")
                        e_tiles[idx] = e_t
                        eng = gr.pick_exp(IW - ti["x0s"], ti["w1"] - ti["x0s"])
                        emit_exp(eng, ps_t, e_t, ti, bi, r)
                        if idx >= LOOKAHEAD:
                            emit_av(idx - LOOKAHEAD)
                    for idx in range(max(0, len(taus) - LOOKAHEAD),
                                     len(taus)):
                        emit_av(idx)

                    # normalize (engine by load balance)
                    zinv = sb_z.tile([128, 4], f32, tag="z",
                                     name=f"z_{bi}_{r}")
                    nc.vector.reciprocal(zinv, pacc[:, :, D])
                    osb = sb_o.tile([128, 4, D], f16, tag="o",
                                    name=f"o_{bi}_{r}")
                    if gr.pick_norm() == "ACT":
                        for ib in range(4):
                            nc.scalar.activation(
                                osb[:, ib, :], pacc[:, ib, 0:D],
                                mybir.ActivationFunctionType.Copy,
                                scale=zinv[:, ib:ib + 1])
                    else:
                        nc.vector.tensor_tensor(
                            out=osb, in0=pacc[:, :, 0:D],
                            in1=zinv.unsqueeze(2).broadcast_to([128, 4, D]),
                            op=mybir.AluOpType.mult)
                    nc.sync.dma_start(out=out[bi][:, r], in_=osb)
    nc.compile()
    return nc


def _host_prep(q, k, v, valid):
    t = np.clip(np.asarray(valid).astype(np.int64), 0, N)
    perm = np.argsort(t, axis=1, kind="stable")
    t_s = np.take_along_axis(t, perm, axis=1)
    q_s = np.take_along_axis(np.asarray(q, np.float32), perm[..., None],
                             axis=1)
    plan = _classify(t_s)

    qT = np.empty((B, DV, N), np.float16)
    qT[:, 0:D] = np.swapaxes(q_s * SQ_A, 1, 2).astype(np.float16)
    qT[:, D] = 128.0
    kT = np.empty((B, DV, N), np.float16)
    kT[:, 0:D] = np.swapaxes(np.asarray(k, np.float32) * SQ_A, 1, 2
                             ).astype(np.float16)
    kT[:, D] = 120.0

    v32 = np.asarray(v, np.float32)
    vwt = np.empty((B, 128, NJ, DV), np.float16)
    vwt[:, :, :, 0:D] = np.swapaxes(
        v32.reshape(B, NJ, 128, D), 1, 2).astype(np.float16)
    vwt[:, :, :, D] = 1.0

    ss = np.zeros((B, N + 1, D), np.float64)
    ss[:, :-1] = np.cumsum(v32[:, ::-1, :].astype(np.float64),
                           axis=1)[:, ::-1, :]
    ssg = np.take_along_axis(ss, t_s[..., None], axis=1)   # [B, N, D]
    cnt = (N - t_s).astype(np.float64)                     # [B, N]
    corr = np.empty((B, N, DV), np.float64)
    corr[:, :, 0:D] = ssg * E6
    corr[:, :, D] = cnt * E6
    corrt = np.ascontiguousarray(
        corr.reshape(B, NI, 4, 128, DV).transpose(0, 3, 1, 2, 4)
    ).astype(np.float16)

    m16v = np.zeros((B, 128, plan.m16_w), np.float16)
    jj = np.arange(128)
    for r in range(NI):
        for ti in plan.taus[r]:
            if ti["m_off"] is None:
                continue
            x0, w1, tau = ti["x0s"], ti["w1"], ti["tau"]
            tloc = t_s[:, r * IW + x0: r * IW + w1]          # [B, w]
            mloc = tloc[:, None, :] > (JW * tau + jj)[None, :, None]
            m16v[:, :, ti["m_off"]:ti["m_off"] + (w1 - x0)] = mloc
    return plan, perm, qT, kT, vwt, corrt, m16v


LAST = {}


def kernel(q, k, v, valid, _trace=False):
    plan, perm, qT, kT, vwt, corrt, m16v = _host_prep(q, k, v, valid)
    nc = _build_program(plan)

    in_maps = []
    for c in range(NCORES):
        sl = slice(c * NB, (c + 1) * NB)
        in_maps.append({
            "qT": np.ascontiguousarray(qT[sl]),
            "kT": np.ascontiguousarray(kT[sl]),
            "vw": np.ascontiguousarray(vwt[sl]),
            "corr": np.ascontiguousarray(corrt[sl]),
            "m16": np.ascontiguousarray(m16v[sl]),
        })
    res = run_bass_kernel_spmd(nc, in_maps, list(range(NCORES)),
                               trace=_trace)
    LAST["res"] = res
    LAST["nc"] = nc

    out = np.empty((B, N, D), np.float32)
    for c in range(NCORES):
        o = res.results[c]["out"]          # [NB, 128, NI, 4, D] fp16
        for bi in range(NB):
            b = c * NB + bi
            o_sorted = o[bi].transpose(1, 2, 0, 3).reshape(N, D)
            out[b, perm[b]] = o_sorted.astype(np.float32)
    return out
